# revision 31
# baseline (speedup 1.0000x reference)
"""MoE task-attention Trainium2 kernel.

Sharding: 8 cores = (batch 0..3) x (sequence half 0..1). Each core computes
everything for its 1024 "own" tokens against the full 2048-token context of
its batch (context order rotated so own tokens come first; attention is
permutation-invariant over context). Outputs are disjoint; tiny aux-loss
partial sums are combined on the host.
"""

import numpy as np
import ml_dtypes
from contextlib import ExitStack

import concourse.bass as bass
import concourse.mybir as mybir
import concourse.tile as tile

# problem constants (hardcoded; kernel.py must be self-contained)
B, N, D = 4, 2048, 1024
E, H, HD, TASKS = 24, 8, 128, 9
SWITCHLOSS, ZLOSS = 0.1, 0.001
R = 1024          # tokens owned per core
C = 2048          # context length
P = 128
DC = D // P       # 8 d-chunks
RT = R // P       # 8 row tiles
CT = C // P       # 16 context tiles
SCALE = HD ** -0.5

f32 = mybir.dt.float32
fp16 = mybir.dt.float16  # all big matmuls run fp16 (1 cyc/row like fp16, ~8x better mantissa)
i16 = mybir.dt.int16
u16 = mybir.dt.uint16

_NC_CACHE = None
_PATCHED = [False]
_SIM_MODE = [False]
_STOP_AFTER = [99]  # debug: truncate kernel after phase N


def _sync_classes():
    nc0 = bass.Bass()
    with nc0.semaphore("s") as s0:
        i = nc0.sync.sem_inc(s0, 1)
        return type(i.ins.sync_info), type(i.ins.sync_info.on_update[0])


def _apply_compat_patches():
    """This container's walrus encodes at most ONE semaphore wait per
    instruction and rejects the EVENT_SEMAPHORE_RANGE_CLEAR ISA op that
    TileContext's kernel tail emits. Patch sem_clear to per-semaphore
    wr-imm updates; extra waits are hoisted onto NOPs by
    _split_multiwait() after the program is built."""
    if _PATCHED[0]:
        return
    SI, SU = _sync_classes()

    import bass_rust as _br
    _rust_sem_clear = vars(_br.RustBassEngine)["sem_clear"]

    def _compat_sem_clear(self, sem):
        if _SIM_MODE[0]:
            return _rust_sem_clear(self, sem)
        nums = list(sem) if isinstance(sem, range) else [getattr(sem, "num", sem)]
        last = None
        for n in nums:
            nop = self.nop()
            nop.ins.sync_info = SI(on_wait=[], on_update=[
                SU(sync_type="semaphore", id=n, update_mode="sem-wr-imm",
                   update_value=0)])
            last = nop
        return last if last is not None else self.nop()

    bass.BassEngine.sem_clear = _compat_sem_clear

    import concourse.bass_interp as bass_interp
    _orig_visit = bass_interp.InstructionExecutor.visit_InstISA

    def _patched_visit(self, instruction, *, reg_snapshot):
        if instruction.isa_opcode == 223:
            d = instruction.ant_dict or {}
            self.pool_library_index = int(d.get("lib_index", 0))
            return
        return _orig_visit(self, instruction, reg_snapshot=reg_snapshot)

    bass_interp.InstructionExecutor.visit_InstISA = _patched_visit

    _orig_mod_visit = bass_interp._visit_InstISA

    def _patched_mod_visit(isa, instruction, core_sim):
        if instruction.isa_opcode == 223:
            d = instruction.ant_dict or {}
            try:
                core_sim.pool_library_index = int(d.get("lib_index", 0))
            except Exception:
                pass
            return
        return _orig_mod_visit(isa, instruction, core_sim)

    bass_interp._visit_InstISA = _patched_mod_visit
    _PATCHED[0] = True


def _split_multiwait(nc):
    SI, _ = _sync_classes()
    ctr = 0
    for f in nc.m.functions:
        for bb in f.blocks:
            il = bb.instructions
            if not any(i.sync_info is not None and len(i.sync_info.on_wait) > 1
                       for i in il):
                continue
            new = []
            for ins in il:
                si = ins.sync_info
                if si is not None and len(si.on_wait) > 1 and ins.engine is not None:
                    waits = list(si.on_wait)
                    for w in waits[:-1]:
                        ctr += 1
                        nop = mybir.InstNoOp(name=f"waitnop_{ctr}", ins=[],
                                             outs=[], engine=ins.engine)
                        nop.sync_info = SI(on_wait=[w], on_update=[])
                        new.append(nop)
                    ins.sync_info = SI(on_wait=[waits[-1]],
                                       on_update=list(si.on_update))
                new.append(ins)
            bb.instructions = new


def _stub_outputs(tc, nc, y, aux):
    with tc.tile_pool(name="stub", bufs=1) as sp:
        z = sp.tile([128, D], mybir.dt.float32)
        nc.vector.memset(z, 0.0)
        for rt in range(RT):
            nc.sync.dma_start(y.ap()[rt * 128:(rt + 1) * 128, :], z)


def _ap(t, offset, pattern):
    return bass.AP(tensor=t.ap().tensor, offset=offset, ap=[list(p) for p in pattern])


def build_bass():
    _apply_compat_patches()
    nc = bass.Bass()

    x_ctx = nc.declare_dram_parameter("x_ctx", [C, D], f32, isOutput=False)
    wg = nc.declare_dram_parameter("wg", [D, E], f32, isOutput=False)
    wq = nc.declare_dram_parameter("wq", [D, E * HD], fp16, isOutput=False)
    wo = nc.declare_dram_parameter("wo", [E * HD, D], fp16, isOutput=False)
    wkv = nc.declare_dram_parameter("wkv", [D, 2 * HD], fp16, isOutput=False)
    bkv = nc.declare_dram_parameter("bkv", [2 * HD], f32, isOutput=False)
    ident_d = nc.declare_dram_parameter("ident", [P, P], f32, isOutput=False)
    consts_d = nc.declare_dram_parameter("consts", [P, 32], f32, isOutput=False)

    y = nc.declare_dram_parameter("y", [R, D], f32, isOutput=True)
    aux = nc.declare_dram_parameter("aux", [P, 64], f32, isOutput=True)

    # internal DRAM staging
    qall_d = nc.dram_tensor("qall_d", [R * E, HD], fp16)      # row r*24+e
    o_d = nc.dram_tensor("o_d", [R * H + P, HD], fp16)        # row h*R+r (contiguous per (h,rt)); row R*H: zeros
    qidx_d = nc.dram_tensor("qidx_d", [R, H], i16)
    cidx_d = nc.dram_tensor("cidx_d", [R, E], i16)

    with tile.TileContext(nc) as tc:
        _body(tc, x_ctx, wg, wq, wo, wkv, bkv, ident_d, consts_d, y, aux, qall_d, o_d, qidx_d, cidx_d)
    if not _SIM_MODE[0]:
        _split_multiwait(nc)
    return nc


def _body(tc, x_ctx, wg, wq, wo, wkv, bkv, ident_d, consts_d, y, aux, qall_d, o_d, qidx_d, cidx_d):
    nc = tc.nc
    Exp = mybir.ActivationFunctionType.Exp
    Ln = mybir.ActivationFunctionType.Ln
    Square = mybir.ActivationFunctionType.Square
    Identity = mybir.ActivationFunctionType.Identity
    ADD = mybir.AluOpType.add
    MULT = mybir.AluOpType.mult

    with ExitStack() as ctx:
        const = ctx.enter_context(tc.tile_pool(name="const", bufs=1))
        persist = ctx.enter_context(tc.tile_pool(name="persist", bufs=1))

        from concourse import library_config
        # raw PSEUDO_INST/LIBRARY_RELOAD_INDEX encoding: the Python
        # load_library() emits an InstPseudoReloadLibraryIndex with no
        # instruction bytes, which this walrus rejects (ISA wrong length).
        nc.gpsimd.isa(
            223,
            {"pseudo_opcode": 2, "lib_index": int(library_config.mlp.index)},
            struct_name="NEURON_ISA_TPB_PSEUDO_LIBRARY_RELOAD_INDEX_STRUCT",
        )

        ident = const.tile([P, P], f32)
        nc.sync.dma_start(ident, ident_d.ap())
        consts = const.tile([P, 32], f32)
        nc.sync.dma_start(consts, consts_d.ap())

        # ---- constants / small weights ----
        wkv_sb = const.tile([P, DC, 2 * HD], fp16)
        nc.sync.dma_start(wkv_sb, wkv.ap().rearrange("(dc p) j -> p dc j", p=P))
        wg_sb = const.tile([P, DC, E], f32)
        nc.sync.dma_start(wg_sb, wg.ap().rearrange("(dc p) e -> p dc e", p=P))
        bkv_sb = const.tile([P, 2], f32)
        nc.sync.dma_start(bkv_sb, bkv.ap().rearrange("(t p) -> p t", p=P))

        # persistent SBUF
        kvT = persist.tile([P, 2, C], fp16)         # [j, {k,v}, c] feature-major k/v
        vT32 = persist.tile([P, C], f32)            # f32 v^T (PE transpose input; 16-bit PSUM is illegal on TRN2)
        v1 = persist.tile([P, CT, HD + 1], fp16)    # [c-part, c-tile, HD|1]
        gates_sb = persist.tile([P, RT, H], f32)    # renormalized gates

        ctx_xt = tc.tile_pool(name="xt_pool", bufs=1)
        xt_pool = ctx_xt.__enter__()
        xt_my = xt_pool.tile([P, DC, R], f32)       # x^T own tokens, f32 (exact gating)
        xt16_my = xt_pool.tile([P, DC, R], fp16)    # fp16 copy for q/kv matmuls

        # ======== phase 1: x^T + k/v projection ========
        with tc.tile_pool(name="p1", bufs=3) as p1, \
             tc.tile_pool(name="p1b", bufs=2) as p1b, \
             tc.tile_pool(name="p1ps", bufs=4, space="PSUM") as p1ps, \
             tc.tile_pool(name="p1kv", bufs=2, space="PSUM") as p1kv:
            for cb in range(4):             # context blocks of 512
                if cb >= 2:
                    xt_blk = p1b.tile([P, DC, 512], fp16, tag="xtblk")
                xrow4 = p1.tile([P, 4, D], f32, tag="xrow")
                nc.sync.dma_start(
                    xrow4, x_ctx.ap()[cb * 512:(cb + 1) * 512, :].rearrange(
                        "(t p) d -> p t d", p=P))
                for t in range(4):
                    for dc in range(DC):
                        ps = p1ps.tile([P, P], f32, tag="tp")
                        nc.tensor.transpose(ps, xrow4[:, t, dc * P:(dc + 1) * P], ident)
                        if cb < 2:
                            nc.vector.tensor_copy(xt_my[:, dc, cb * 512 + t * P: cb * 512 + (t + 1) * P], ps)
                            nc.scalar.copy(xt16_my[:, dc, cb * 512 + t * P: cb * 512 + (t + 1) * P], ps)
                        else:
                            if t % 2 == 0:
                                nc.vector.tensor_copy(xt_blk[:, dc, t * P:(t + 1) * P], ps)
                            else:
                                nc.scalar.copy(xt_blk[:, dc, t * P:(t + 1) * P], ps)
                src3 = xt16_my[:, :, cb * 512:(cb + 1) * 512] if cb < 2 else xt_blk
                for jt in range(2):
                    pkv = p1kv.tile([P, 512], f32, tag="kv")
                    for dc in range(DC):
                        nc.tensor.matmul(
                            pkv,
                            wkv_sb[:, dc, jt * P:(jt + 1) * P],
                            src3[:, dc, :],
                            start=(dc == 0), stop=(dc == DC - 1),
                        )
                    # add bias while copying out
                    nc.scalar.activation(
                        kvT[:, jt, cb * 512:(cb + 1) * 512], pkv, Identity,
                        bias=bkv_sb[:, jt:jt + 1], scale=1.0,
                    )
                    if jt == 1:
                        nc.scalar.activation(
                            vT32[:, cb * 512:(cb + 1) * 512], pkv, Identity,
                            bias=bkv_sb[:, jt:jt + 1], scale=1.0,
                        )

        # ======== phase 2: v row-major (+ones col), zero rows of o_d ========
        with tc.tile_pool(name="p2ps", bufs=4, space="PSUM") as p2ps, \
             tc.tile_pool(name="p2", bufs=2) as p2:
            for cc in range(CT):
                ps = p2ps.tile([P, P], f32, tag="vt")
                nc.tensor.transpose(ps, vT32[:, cc * P:(cc + 1) * P], ident)
                nc.vector.tensor_copy(v1[:, cc, 0:HD], ps)
            nc.vector.memset(v1[:, :, HD:HD + 1], 1.0)
            zrow = p2.tile([P, HD], fp16)
            nc.vector.memset(zrow, 0.0)
            nc.sync.dma_start(_ap(o_d, R * H * HD, [[HD, P], [1, HD]]), zrow)

        if _STOP_AFTER[0] < 3:
            _stub_outputs(tc, nc, y, aux)
            ctx_xt.__exit__(None, None, None)
            return
        # ======== phase 3+4: gating + q_all projection, per row tile ========
        ctx_wq = tc.tile_pool(name="wq_pool", bufs=1)
        wq_pool = ctx_wq.__enter__()
        wq_sb = wq_pool.tile([P, DC, E * HD], fp16)
        nc.scalar.dma_start(wq_sb, wq.ap().rearrange("(dc p) n -> p dc n", p=P))

        qidx_all = persist.tile([P, RT, H], i16)
        cidx_all = persist.tile([P, RT, E], i16)
        freq_acc = persist.tile([P, E], f32)
        pacc = persist.tile([P, E], f32)
        zacc = persist.tile([P, 1], f32)
        nc.vector.memset(freq_acc, 0.0)
        nc.vector.memset(pacc, 0.0)
        nc.vector.memset(zacc, 0.0)

        with tc.tile_pool(name="p3", bufs=3) as p3, \
             tc.tile_pool(name="p3s", bufs=2) as p3s, \
             tc.tile_pool(name="p3q", bufs=2, space="PSUM") as p3q, \
             tc.tile_pool(name="p3l", bufs=2, space="PSUM") as p3l:
            for rt in range(RT):
                ps_l = p3l.tile([P, E], f32, tag="logit")
                ps_qh = []
                for hb in range(2):
                    t = p3q.tile([P, E * HD // 2], f32, tag="qall", name=f"psq{rt}_{hb}")
                    ps_qh.append(t)
                for dc in range(DC):
                    nc.tensor.matmul(ps_l, xt_my[:, dc, rt * P:(rt + 1) * P],
                                     wg_sb[:, dc, :],
                                     start=(dc == 0), stop=(dc == DC - 1))
                    lhs = xt16_my[:, dc, rt * P:(rt + 1) * P]
                    for nb in range(6):
                        nc.tensor.matmul(
                            ps_qh[nb // 3][:, (nb % 3) * 512:(nb % 3 + 1) * 512], lhs,
                            wq_sb[:, dc, nb * 512:(nb + 1) * 512],
                            start=(dc == 0), stop=(dc == DC - 1))

                # gating chain
                expl = p3s.tile([P, E], f32, tag="expl")
                sume = p3s.tile([P, 1], f32, tag="sume")
                nc.scalar.activation(expl, ps_l, Exp, accum_out=sume)
                rsum = p3s.tile([P, 1], f32, tag="rsum")
                nc.vector.reciprocal(rsum, sume)
                probs = p3s.tile([P, E], f32, tag="probs")
                nc.vector.tensor_scalar_mul(probs, expl, rsum)
                nc.vector.tensor_tensor(pacc, pacc, probs, ADD)
                # z-loss partial: (ln sum exp)^2
                lse = p3s.tile([P, 1], f32, tag="lse")
                nc.scalar.activation(lse, sume, Ln)
                lse2 = p3s.tile([P, 1], f32, tag="lse2")
                nc.scalar.activation(lse2, lse, Square)
                nc.vector.tensor_tensor(zacc, zacc, lse2, ADD)

                # top-8
                topg = p3s.tile([P, H], f32, tag="topg")
                topi = p3s.tile([P, H], u16, tag="topi")
                nc.vector.max(topg, probs)
                nc.vector.max_index(topi, topg, probs)
                gsum = p3s.tile([P, 1], f32, tag="gsum")
                nc.vector.tensor_reduce(gsum, topg, mybir.AxisListType.X, ADD)
                rg = p3s.tile([P, 1], f32, tag="rg")
                nc.vector.reciprocal(rg, gsum)
                nc.vector.tensor_scalar_mul(gates_sb[:, rt, :], topg, rg)

                # qidx = r*24 + topi  (f32 integer arithmetic, cast to i16)
                topi_f = p3s.tile([P, H], f32, tag="topif")
                nc.vector.tensor_copy(topi_f, topi)
                qidx_f = p3s.tile([P, H], f32, tag="qidxf")
                nc.vector.tensor_scalar(qidx_f, topi_f, consts[:, 24:25],
                                        float(rt * P * E), ADD, ADD)
                nc.vector.tensor_copy(qidx_all[:, rt, :], qidx_f)

                # inverse map inv[r,e] = h+1 if top_i[r,h]==e else 0
                inv_f = p3s.tile([P, E], f32, tag="invf")
                nc.vector.memset(inv_f, 0.0)
                for h in range(H):
                    cmp = p3s.tile([P, E], f32, tag="cmp")
                    nc.vector.tensor_scalar(cmp, consts[:, 0:E],
                                            topi_f[:, h:h + 1], None,
                                            mybir.AluOpType.is_equal)
                    nc.vector.scalar_tensor_tensor(
                        inv_f, cmp, float(h + 1), inv_f,
                        op0=MULT, op1=ADD)
                # freq indicator: inv != 0
                ind = p3s.tile([P, E], f32, tag="ind")
                nc.vector.tensor_scalar(ind, inv_f, 0.0, None,
                                        mybir.AluOpType.is_gt)
                nc.vector.tensor_tensor(freq_acc, freq_acc, ind, ADD)

                # cidx = (inv-1)*R + r when selected else R*H (zero row)
                # consts[:,26] = p (partition index); add rt*P for global r
                cidx_f = p3s.tile([P, E], f32, tag="cidxf")
                nc.vector.tensor_scalar(cidx_f, inv_f, float(R), consts[:, 26:27],
                                        mybir.AluOpType.mult, ADD)
                nc.vector.tensor_scalar(cidx_f, cidx_f, float(rt * P - R), None, ADD)
                nc.vector.memset(cidx_all[:, rt, :], float(R * H))
                cidxv = p3s.tile([P, E], i16, tag="cidxv")
                nc.vector.tensor_copy(cidxv, cidx_f)
                ind16 = p3s.tile([P, E], i16, tag="ind16")
                nc.vector.tensor_copy(ind16, ind)
                nc.vector.copy_predicated(cidx_all[:, rt, :], ind16, cidxv)

                # spill q_all (two psum halves -> one fp16 tile)
                qall_sb = p3.tile([P, E * HD], fp16, tag="qallsb")
                nc.vector.tensor_copy(qall_sb[:, :E * HD // 2], ps_qh[0])
                nc.vector.tensor_copy(qall_sb[:, E * HD // 2:], ps_qh[1])
                nc.scalar.dma_start(
                    _ap(qall_d, rt * P * E * HD, [[E * HD, P], [1, E * HD]]),
                    qall_sb)

        nc.sync.dma_start(
            qidx_d.ap().rearrange("(rt p) h -> p rt h", p=P), qidx_all)
        nc.scalar.dma_start(
            cidx_d.ap().rearrange("(rt p) e -> p rt e", p=P), cidx_all)

        ctx_wq.__exit__(None, None, None)
        ctx_xt.__exit__(None, None, None)

        # aux out
        with tc.tile_pool(name="paux", bufs=1) as paux:
            aux_sb = paux.tile([P, 64], f32)
            nc.vector.memset(aux_sb, 0.0)
            nc.vector.tensor_copy(aux_sb[:, 0:E], freq_acc)
            nc.vector.tensor_copy(aux_sb[:, E:2 * E], pacc)
            nc.vector.tensor_copy(aux_sb[:, 2 * E:2 * E + 1], zacc)
            nc.sync.dma_start(aux.ap()[:, :], aux_sb)

        if _STOP_AFTER[0] < 5:
            _stub_outputs(tc, nc, y, aux)
            return
        # ======== phase 5: gather q, build q^T ========
        ctx_qt = tc.tile_pool(name="qt_pool", bufs=1)
        qt_pool = ctx_qt.__enter__()
        qT_sb = qt_pool.tile([P, H, R], fp16)       # gathered q^T per head
        with tc.tile_pool(name="p5", bufs=1) as p5, \
             tc.tile_pool(name="p5ps", bufs=4, space="PSUM") as p5ps:
            qidx_w = p5.tile([P, R * H // 16], i16)   # wrapped, replicated
            for g in range(8):
                nc.sync.dma_start(
                    qidx_w[g * 16:(g + 1) * 16, :],
                    _ap(qidx_d, 0, [[H, 16], [1, H], [P * H, RT], [16 * H, 8]]))
            q_sel = p5.tile([P, R * H // P, HD], fp16)
            for blk in range(H):
                nc.gpsimd.dma_gather(
                    q_sel[:, blk * (R // P):(blk + 1) * (R // P), :],
                    qall_d.ap(), qidx_w[:, blk * (R // 16):(blk + 1) * (R // 16)],
                    num_idxs=R, num_idxs_reg=R, elem_size=HD)
            with tc.tile_pool(name="p5b", bufs=3) as p5b:
                for h in range(H):
                    for rt in range(RT):
                        q32 = p5b.tile([P, P], f32, tag="q32")
                        nc.vector.tensor_copy(q32, q_sel[:, h * RT + rt, :])
                        ps = p5ps.tile([P, P], f32, tag="qtp")
                        nc.tensor.transpose(ps, q32, ident)
                        nc.vector.tensor_copy(qT_sb[:, h, rt * P:(rt + 1) * P], ps)

        if _STOP_AFTER[0] < 6:
            _stub_outputs(tc, nc, y, aux)
            ctx_qt.__exit__(None, None, None)
            return
        # ======== phase 6: attention ========
        with tc.tile_pool(name="p6e", bufs=2) as p6e, \
             tc.tile_pool(name="p6s", bufs=2) as p6s, \
             tc.tile_pool(name="p6ps", bufs=4, space="PSUM") as p6ps, \
             tc.tile_pool(name="p6po", bufs=2, space="PSUM") as p6po:
            for h in range(H):
                expS = p6e.tile([P, CT, R], fp16, tag="expS")
                for rb in range(2):
                    for ct in range(CT):
                        ps_s = p6ps.tile([P, 512], f32, tag="s")
                        nc.tensor.matmul(
                            ps_s,
                            kvT[:, 0, ct * P:(ct + 1) * P],
                            qT_sb[:, h, rb * 512:(rb + 1) * 512],
                            start=True, stop=True)
                        nc.scalar.activation(
                            expS[:, ct, rb * 512:(rb + 1) * 512], ps_s, Exp,
                            scale=SCALE)
                for rt in range(RT):
                    ps_o = p6po.tile([P, HD + 1], f32, tag="o")
                    for ct in range(CT):
                        nc.tensor.matmul(
                            ps_o, expS[:, ct, rt * P:(rt + 1) * P],
                            v1[:, ct, :],
                            start=(ct == 0), stop=(ct == CT - 1))
                    rd = p6s.tile([P, 1], f32, tag="rd")
                    nc.vector.reciprocal(rd, ps_o[:, HD:HD + 1])
                    g2 = p6s.tile([P, 1], f32, tag="g2")
                    nc.vector.tensor_tensor(g2, gates_sb[:, rt, h:h + 1], rd, MULT)
                    o_sb = p6s.tile([P, HD], fp16, tag="osb")
                    nc.vector.tensor_scalar_mul(o_sb, ps_o[:, 0:HD], g2)
                    nc.sync.dma_start(
                        _ap(o_d, (h * R + rt * P) * HD, [[HD, P], [1, HD]]),
                        o_sb)

        ctx_qt.__exit__(None, None, None)
        if _STOP_AFTER[0] < 7:
            _stub_outputs(tc, nc, y, aux)
            return

        # ======== phase 7+8: gather c^T, combine ========
        with tc.tile_pool(name="p8w", bufs=1) as p8w, \
             tc.tile_pool(name="p8", bufs=2) as p8, \
             tc.tile_pool(name="p8ps", bufs=2, space="PSUM") as p8ps:
            cidx_w = p8w.tile([P, R * E // 16], i16)
            for g in range(8):
                nc.sync.dma_start(
                    cidx_w[g * 16:(g + 1) * 16, :],
                    _ap(cidx_d, 0, [[E, 16], [1, E], [P * E, RT], [16 * E, 8]]))
            cT = p8w.tile([P, 1, R * E], fp16)
            # column order j = rt*3072 + e*128 + p so combine(rt) only needs
            # its own 4 gather blocks (768 idxs each; <=~1024 descriptors fit
            # the SWDGE carveout)
            GCH = 768
            wo_sb = p8w.tile([P, E, D], fp16)
            nc.scalar.dma_start(wo_sb, wo.ap().rearrange("(e p) n -> p e n", p=P))
            for blk in range(R * E // GCH):
                nc.gpsimd.dma_gather(
                    cT[:, :, blk * GCH:(blk + 1) * GCH],
                    o_d.ap(), cidx_w[:, blk * (GCH // 16):(blk + 1) * (GCH // 16)],
                    num_idxs=GCH, num_idxs_reg=GCH, elem_size=HD, transpose=True)

            # two expert halves: first half's matmuls only need the first 16
            # gather blocks, so they overlap the remaining gathers
            ADD_ = mybir.AluOpType.add
            for rt in range(RT):
                y_sb = p8.tile([P, D], f32, tag="ysb")
                for eh in range(2):
                    ps_y = []
                    for db in range(2):
                        t = p8ps.tile([P, 512], f32, tag=f"y{db}", name=f"psy{eh}{db}")
                        ps_y.append(t)
                    for ei in range(E // 2):
                        e = eh * (E // 2) + ei
                        lhs = cT[:, 0, e * R + rt * P: e * R + (rt + 1) * P]
                        for db in range(2):
                            nc.tensor.matmul(
                                ps_y[db], lhs, wo_sb[:, e, db * 512:(db + 1) * 512],
                                start=(ei == 0), stop=(ei == E // 2 - 1))
                    for db in range(2):
                        if eh == 0:
                            nc.vector.tensor_copy(y_sb[:, db * 512:(db + 1) * 512], ps_y[db])
                        else:
                            nc.vector.tensor_tensor(
                                y_sb[:, db * 512:(db + 1) * 512],
                                y_sb[:, db * 512:(db + 1) * 512], ps_y[db], ADD_)
                nc.scalar.dma_start(y.ap()[rt * P:(rt + 1) * P, :], y_sb)


def _get_nc():
    global _NC_CACHE
    if _NC_CACHE is None:
        _NC_CACHE = build_bass()
    return _NC_CACHE


def _ident_const():
    return np.eye(P, dtype=np.float32)


def _consts_const():
    c = np.zeros((P, 32), dtype=np.float32)
    c[:, 0:E] = np.arange(E, dtype=np.float32)[None, :]
    c[:, 24] = np.arange(P, dtype=np.float32) * E
    c[:, 25] = np.arange(P, dtype=np.float32) * 9
    c[:, 26] = np.arange(P, dtype=np.float32)
    return c


def kernel(x, task_bh, Wg, Wq, Wo, Wkv, bkv):
    from concourse.bass_utils import run_bass_kernel_spmd

    x = np.asarray(x, dtype=np.float32)
    task = np.asarray(task_bh).astype(np.int64)
    Wg = np.asarray(Wg, dtype=np.float32)
    Wq = np.asarray(Wq, dtype=np.float32)
    Wo = np.asarray(Wo, dtype=np.float32)
    Wkv = np.asarray(Wkv, dtype=np.float32)
    bkv = np.asarray(bkv, dtype=np.float32)

    wq_flat = np.ascontiguousarray(Wq.transpose(1, 0, 2).reshape(D, E * HD)).astype(np.float16)
    wo_bf = np.ascontiguousarray(Wo.reshape(E * HD, D)).astype(np.float16)

    in_maps = []
    for core in range(8):
        b, half = core // 2, core % 2
        xb = x[b]
        x_rot = np.ascontiguousarray(
            np.concatenate([xb[half * R:(half + 1) * R],
                            xb[(1 - half) * R:(2 - half) * R]], axis=0))
        in_maps.append({
            "x_ctx": x_rot,
            "wg": np.ascontiguousarray(Wg[task[b]]),
            "wq": wq_flat,
            "wo": wo_bf,
            "wkv": Wkv.astype(np.float16),
            "bkv": bkv,
            "ident": _ident_const(),
            "consts": _consts_const(),
        })

    nc = _get_nc()
    res = run_bass_kernel_spmd(nc, in_maps, core_ids=list(range(8)))
    outs = res.results

    yfull = np.empty((B, N, D), dtype=np.float32)
    freq_cnt = np.zeros(E, dtype=np.float64)
    psum = np.zeros(E, dtype=np.float64)
    zsum = 0.0
    for core in range(8):
        b, half = core // 2, core % 2
        yfull[b, half * R:(half + 1) * R] = outs[core]["y"]
        a = outs[core]["aux"]
        freq_cnt += a[:, 0:E].sum(axis=0)
        psum += a[:, E:2 * E].sum(axis=0)
        zsum += a[:, 2 * E].sum()
    denom = B * N
    freq = freq_cnt / denom
    pmean = psum / denom
    switch = SWITCHLOSS * E * float(np.dot(freq, pmean))
    zl = ZLOSS * zsum / denom
    aux_loss = np.float32(switch + zl)
    return yfull, aux_loss


# revision 37
# speedup vs baseline: 1.0202x; 1.0202x over previous
"""MoE task-attention Trainium2 kernel.

Sharding: 8 cores = (batch 0..3) x (sequence half 0..1). Each core computes
everything for its 1024 "own" tokens against the full 2048-token context of
its batch (context order rotated so own tokens come first; attention is
permutation-invariant over context). Outputs are disjoint; tiny aux-loss
partial sums are combined on the host.
"""

import numpy as np
import ml_dtypes
from contextlib import ExitStack

import concourse.bass as bass
import concourse.mybir as mybir
import concourse.tile as tile

# problem constants (hardcoded; kernel.py must be self-contained)
B, N, D = 4, 2048, 1024
E, H, HD, TASKS = 24, 8, 128, 9
SWITCHLOSS, ZLOSS = 0.1, 0.001
R = 1024          # tokens owned per core
C = 2048          # context length
P = 128
DC = D // P       # 8 d-chunks
RT = R // P       # 8 row tiles
CT = C // P       # 16 context tiles
SCALE = HD ** -0.5

f32 = mybir.dt.float32
fp16 = mybir.dt.float16  # all big matmuls run fp16 (1 cyc/row like fp16, ~8x better mantissa)
i16 = mybir.dt.int16
u16 = mybir.dt.uint16

_NC_CACHE = None
_PATCHED = [False]
_SIM_MODE = [False]
_STOP_AFTER = [99]  # debug: truncate kernel after phase N


def _sync_classes():
    nc0 = bass.Bass()
    with nc0.semaphore("s") as s0:
        i = nc0.sync.sem_inc(s0, 1)
        return type(i.ins.sync_info), type(i.ins.sync_info.on_update[0])


def _apply_compat_patches():
    """This container's walrus encodes at most ONE semaphore wait per
    instruction and rejects the EVENT_SEMAPHORE_RANGE_CLEAR ISA op that
    TileContext's kernel tail emits. Patch sem_clear to per-semaphore
    wr-imm updates; extra waits are hoisted onto NOPs by
    _split_multiwait() after the program is built."""
    if _PATCHED[0]:
        return
    SI, SU = _sync_classes()

    import bass_rust as _br
    _rust_sem_clear = vars(_br.RustBassEngine)["sem_clear"]

    def _compat_sem_clear(self, sem):
        if _SIM_MODE[0]:
            return _rust_sem_clear(self, sem)
        nums = list(sem) if isinstance(sem, range) else [getattr(sem, "num", sem)]
        last = None
        for n in nums:
            nop = self.nop()
            nop.ins.sync_info = SI(on_wait=[], on_update=[
                SU(sync_type="semaphore", id=n, update_mode="sem-wr-imm",
                   update_value=0)])
            last = nop
        return last if last is not None else self.nop()

    bass.BassEngine.sem_clear = _compat_sem_clear

    import concourse.bass_interp as bass_interp
    _orig_visit = bass_interp.InstructionExecutor.visit_InstISA

    def _patched_visit(self, instruction, *, reg_snapshot):
        if instruction.isa_opcode == 223:
            d = instruction.ant_dict or {}
            self.pool_library_index = int(d.get("lib_index", 0))
            return
        return _orig_visit(self, instruction, reg_snapshot=reg_snapshot)

    bass_interp.InstructionExecutor.visit_InstISA = _patched_visit

    _orig_mod_visit = bass_interp._visit_InstISA

    def _patched_mod_visit(isa, instruction, core_sim):
        if instruction.isa_opcode == 223:
            d = instruction.ant_dict or {}
            try:
                core_sim.pool_library_index = int(d.get("lib_index", 0))
            except Exception:
                pass
            return
        return _orig_mod_visit(isa, instruction, core_sim)

    bass_interp._visit_InstISA = _patched_mod_visit
    _PATCHED[0] = True


def _split_multiwait(nc):
    SI, _ = _sync_classes()
    ctr = 0
    for f in nc.m.functions:
        for bb in f.blocks:
            il = bb.instructions
            if not any(i.sync_info is not None and len(i.sync_info.on_wait) > 1
                       for i in il):
                continue
            new = []
            for ins in il:
                si = ins.sync_info
                if si is not None and len(si.on_wait) > 1 and ins.engine is not None:
                    waits = list(si.on_wait)
                    for w in waits[:-1]:
                        ctr += 1
                        nop = mybir.InstNoOp(name=f"waitnop_{ctr}", ins=[],
                                             outs=[], engine=ins.engine)
                        nop.sync_info = SI(on_wait=[w], on_update=[])
                        new.append(nop)
                    ins.sync_info = SI(on_wait=[waits[-1]],
                                       on_update=list(si.on_update))
                new.append(ins)
            bb.instructions = new


def _stub_outputs(tc, nc, y, aux):
    with tc.tile_pool(name="stub", bufs=1) as sp:
        z = sp.tile([128, D], mybir.dt.float32)
        nc.vector.memset(z, 0.0)
        for rt in range(RT):
            nc.sync.dma_start(y.ap()[rt * 128:(rt + 1) * 128, :], z)


def _ap(t, offset, pattern):
    return bass.AP(tensor=t.ap().tensor, offset=offset, ap=[list(p) for p in pattern])


def build_bass():
    _apply_compat_patches()
    nc = bass.Bass()

    x_ctx = nc.declare_dram_parameter("x_ctx", [C, D], f32, isOutput=False)
    wg = nc.declare_dram_parameter("wg", [D, E], f32, isOutput=False)
    wq = nc.declare_dram_parameter("wq", [D, E * HD], fp16, isOutput=False)
    wo = nc.declare_dram_parameter("wo", [E * HD, D], fp16, isOutput=False)
    wkv = nc.declare_dram_parameter("wkv", [D, 2 * HD], fp16, isOutput=False)
    bkv = nc.declare_dram_parameter("bkv", [2 * HD], f32, isOutput=False)
    ident_d = nc.declare_dram_parameter("ident", [P, P], f32, isOutput=False)
    consts_d = nc.declare_dram_parameter("consts", [P, 32], f32, isOutput=False)

    y = nc.declare_dram_parameter("y", [R, D], f32, isOutput=True)
    aux = nc.declare_dram_parameter("aux", [P, 64], f32, isOutput=True)

    # internal DRAM staging
    qall_d = nc.dram_tensor("qall_d", [R * E, HD], fp16)      # row r*24+e
    o_d = nc.dram_tensor("o_d", [R * H + P, HD], fp16)        # row h*R+r (contiguous per (h,rt)); row R*H: zeros
    qidx_d = nc.dram_tensor("qidx_d", [R, H], i16)
    cidx_d = nc.dram_tensor("cidx_d", [R, E], i16)

    with tile.TileContext(nc) as tc:
        _body(tc, x_ctx, wg, wq, wo, wkv, bkv, ident_d, consts_d, y, aux, qall_d, o_d, qidx_d, cidx_d)
    if not _SIM_MODE[0]:
        _split_multiwait(nc)
    return nc


def _body(tc, x_ctx, wg, wq, wo, wkv, bkv, ident_d, consts_d, y, aux, qall_d, o_d, qidx_d, cidx_d):
    nc = tc.nc
    Exp = mybir.ActivationFunctionType.Exp
    Ln = mybir.ActivationFunctionType.Ln
    Square = mybir.ActivationFunctionType.Square
    Identity = mybir.ActivationFunctionType.Identity
    ADD = mybir.AluOpType.add
    MULT = mybir.AluOpType.mult

    with ExitStack() as ctx:
        const = ctx.enter_context(tc.tile_pool(name="const", bufs=1))
        persist = ctx.enter_context(tc.tile_pool(name="persist", bufs=1))

        from concourse import library_config
        # raw PSEUDO_INST/LIBRARY_RELOAD_INDEX encoding: the Python
        # load_library() emits an InstPseudoReloadLibraryIndex with no
        # instruction bytes, which this walrus rejects (ISA wrong length).
        nc.gpsimd.isa(
            223,
            {"pseudo_opcode": 2, "lib_index": int(library_config.mlp.index)},
            struct_name="NEURON_ISA_TPB_PSEUDO_LIBRARY_RELOAD_INDEX_STRUCT",
        )

        ident = const.tile([P, P], f32)
        nc.sync.dma_start(ident, ident_d.ap())
        consts = const.tile([P, 32], f32)
        nc.sync.dma_start(consts, consts_d.ap())

        # ---- constants / small weights ----
        wkv_sb = const.tile([P, DC, 2 * HD], fp16)
        nc.sync.dma_start(wkv_sb, wkv.ap().rearrange("(dc p) j -> p dc j", p=P))
        wg_sb = const.tile([P, DC, E], f32)
        nc.sync.dma_start(wg_sb, wg.ap().rearrange("(dc p) e -> p dc e", p=P))
        bkv_sb = const.tile([P, 2], f32)
        nc.sync.dma_start(bkv_sb, bkv.ap().rearrange("(t p) -> p t", p=P))

        # persistent SBUF
        kvT = persist.tile([P, 2, C], fp16)         # [j, {k,v}, c] feature-major k/v
        vT32 = persist.tile([P, C], f32)            # f32 v^T (PE transpose input; 16-bit PSUM is illegal on TRN2)
        v1 = persist.tile([P, CT, HD + 1], fp16)    # [c-part, c-tile, HD|1]
        gates_sb = persist.tile([P, RT, H], f32)    # renormalized gates

        ctx_xt = tc.tile_pool(name="xt_pool", bufs=1)
        xt_pool = ctx_xt.__enter__()
        xt_my = xt_pool.tile([P, DC, R], f32)       # x^T own tokens, f32 (exact gating)
        xt16_my = xt_pool.tile([P, DC, R], fp16)    # fp16 copy for q/kv matmuls

        # ======== phase 1: x^T + k/v projection ========
        with tc.tile_pool(name="p1", bufs=3) as p1, \
             tc.tile_pool(name="p1b", bufs=2) as p1b, \
             tc.tile_pool(name="p1ps", bufs=4, space="PSUM") as p1ps, \
             tc.tile_pool(name="p1kv", bufs=2, space="PSUM") as p1kv:
            for cb in range(4):             # context blocks of 512
                if cb >= 2:
                    xt_blk = p1b.tile([P, DC, 512], fp16, tag="xtblk")
                xrow4 = p1.tile([P, 4, D], f32, tag="xrow")
                nc.sync.dma_start(
                    xrow4, x_ctx.ap()[cb * 512:(cb + 1) * 512, :].rearrange(
                        "(t p) d -> p t d", p=P))
                for t in range(4):
                    for dc in range(DC):
                        ps = p1ps.tile([P, P], f32, tag="tp")
                        nc.tensor.transpose(ps, xrow4[:, t, dc * P:(dc + 1) * P], ident)
                        if cb < 2:
                            nc.vector.tensor_copy(xt_my[:, dc, cb * 512 + t * P: cb * 512 + (t + 1) * P], ps)
                            nc.scalar.copy(xt16_my[:, dc, cb * 512 + t * P: cb * 512 + (t + 1) * P], ps)
                        else:
                            if t % 2 == 0:
                                nc.vector.tensor_copy(xt_blk[:, dc, t * P:(t + 1) * P], ps)
                            else:
                                nc.scalar.copy(xt_blk[:, dc, t * P:(t + 1) * P], ps)
                src3 = xt16_my[:, :, cb * 512:(cb + 1) * 512] if cb < 2 else xt_blk
                for jt in range(2):
                    pkv = p1kv.tile([P, 512], f32, tag="kv")
                    for dc in range(DC):
                        nc.tensor.matmul(
                            pkv,
                            wkv_sb[:, dc, jt * P:(jt + 1) * P],
                            src3[:, dc, :],
                            start=(dc == 0), stop=(dc == DC - 1),
                        )
                    # add bias while copying out
                    nc.scalar.activation(
                        kvT[:, jt, cb * 512:(cb + 1) * 512], pkv, Identity,
                        bias=bkv_sb[:, jt:jt + 1], scale=1.0,
                    )
                    if jt == 1:
                        nc.scalar.activation(
                            vT32[:, cb * 512:(cb + 1) * 512], pkv, Identity,
                            bias=bkv_sb[:, jt:jt + 1], scale=1.0,
                        )

        # ======== phase 2: v row-major (+ones col), zero rows of o_d ========
        with tc.tile_pool(name="p2ps", bufs=4, space="PSUM") as p2ps, \
             tc.tile_pool(name="p2", bufs=2) as p2:
            for cc in range(CT):
                ps = p2ps.tile([P, P], f32, tag="vt")
                nc.tensor.transpose(ps, vT32[:, cc * P:(cc + 1) * P], ident)
                nc.vector.tensor_copy(v1[:, cc, 0:HD], ps)
            nc.vector.memset(v1[:, :, HD:HD + 1], 1.0)
            zrow = p2.tile([P, HD], fp16)
            nc.vector.memset(zrow, 0.0)
            nc.sync.dma_start(_ap(o_d, R * H * HD, [[HD, P], [1, HD]]), zrow)

        if _STOP_AFTER[0] < 3:
            _stub_outputs(tc, nc, y, aux)
            ctx_xt.__exit__(None, None, None)
            return
        # ======== phase 3+4: gating + q_all projection, per row tile ========
        ctx_wq = tc.tile_pool(name="wq_pool", bufs=1)
        wq_pool = ctx_wq.__enter__()
        wq_sb = wq_pool.tile([P, DC, E * HD], fp16)
        nc.scalar.dma_start(wq_sb, wq.ap().rearrange("(dc p) n -> p dc n", p=P))

        qidx_all = persist.tile([P, RT, H], i16)
        cidx_all = persist.tile([P, RT, E], i16)
        freq_acc = persist.tile([P, E], f32)
        pacc = persist.tile([P, E], f32)
        zacc = persist.tile([P, 1], f32)
        nc.vector.memset(freq_acc, 0.0)
        nc.vector.memset(pacc, 0.0)
        nc.vector.memset(zacc, 0.0)

        with tc.tile_pool(name="p3", bufs=3) as p3, \
             tc.tile_pool(name="p3s", bufs=2) as p3s, \
             tc.tile_pool(name="p3q", bufs=2, space="PSUM") as p3q, \
             tc.tile_pool(name="p3l", bufs=2, space="PSUM") as p3l:
            for rt in range(RT):
                ps_l = p3l.tile([P, E], f32, tag="logit")
                ps_qh = []
                for hb in range(2):
                    t = p3q.tile([P, E * HD // 2], f32, tag="qall", name=f"psq{rt}_{hb}")
                    ps_qh.append(t)
                for dc in range(DC):
                    nc.tensor.matmul(ps_l, xt_my[:, dc, rt * P:(rt + 1) * P],
                                     wg_sb[:, dc, :],
                                     start=(dc == 0), stop=(dc == DC - 1))
                    lhs = xt16_my[:, dc, rt * P:(rt + 1) * P]
                    for nb in range(6):
                        nc.tensor.matmul(
                            ps_qh[nb // 3][:, (nb % 3) * 512:(nb % 3 + 1) * 512], lhs,
                            wq_sb[:, dc, nb * 512:(nb + 1) * 512],
                            start=(dc == 0), stop=(dc == DC - 1))

                # gating chain
                expl = p3s.tile([P, E], f32, tag="expl")
                sume = p3s.tile([P, 1], f32, tag="sume")
                nc.scalar.activation(expl, ps_l, Exp, accum_out=sume)
                rsum = p3s.tile([P, 1], f32, tag="rsum")
                nc.vector.reciprocal(rsum, sume)
                probs = p3s.tile([P, E], f32, tag="probs")
                nc.vector.tensor_scalar_mul(probs, expl, rsum)
                nc.vector.tensor_tensor(pacc, pacc, probs, ADD)
                # z-loss partial: (ln sum exp)^2
                lse = p3s.tile([P, 1], f32, tag="lse")
                nc.scalar.activation(lse, sume, Ln)
                lse2 = p3s.tile([P, 1], f32, tag="lse2")
                nc.scalar.activation(lse2, lse, Square)
                nc.vector.tensor_tensor(zacc, zacc, lse2, ADD)

                # top-8
                topg = p3s.tile([P, H], f32, tag="topg")
                topi = p3s.tile([P, H], u16, tag="topi")
                nc.vector.max(topg, probs)
                nc.vector.max_index(topi, topg, probs)
                gsum = p3s.tile([P, 1], f32, tag="gsum")
                nc.vector.tensor_reduce(gsum, topg, mybir.AxisListType.X, ADD)
                rg = p3s.tile([P, 1], f32, tag="rg")
                nc.vector.reciprocal(rg, gsum)
                nc.vector.tensor_scalar_mul(gates_sb[:, rt, :], topg, rg)

                # qidx = r*24 + topi  (f32 integer arithmetic, cast to i16)
                topi_f = p3s.tile([P, H], f32, tag="topif")
                nc.vector.tensor_copy(topi_f, topi)
                qidx_f = p3s.tile([P, H], f32, tag="qidxf")
                nc.vector.tensor_scalar(qidx_f, topi_f, consts[:, 24:25],
                                        float(rt * P * E), ADD, ADD)
                nc.vector.tensor_copy(qidx_all[:, rt, :], qidx_f)

                # inverse map inv[r,e] = h+1 if top_i[r,h]==e else 0
                inv_f = p3s.tile([P, E], f32, tag="invf")
                nc.vector.memset(inv_f, 0.0)
                for h in range(H):
                    cmp = p3s.tile([P, E], f32, tag="cmp")
                    nc.vector.tensor_scalar(cmp, consts[:, 0:E],
                                            topi_f[:, h:h + 1], None,
                                            mybir.AluOpType.is_equal)
                    nc.vector.scalar_tensor_tensor(
                        inv_f, cmp, float(h + 1), inv_f,
                        op0=MULT, op1=ADD)
                # freq indicator: inv != 0
                ind = p3s.tile([P, E], f32, tag="ind")
                nc.vector.tensor_scalar(ind, inv_f, 0.0, None,
                                        mybir.AluOpType.is_gt)
                nc.vector.tensor_tensor(freq_acc, freq_acc, ind, ADD)

                # cidx = (inv-1)*R + r when selected else R*H (zero row)
                # consts[:,26] = p (partition index); add rt*P for global r
                cidx_f = p3s.tile([P, E], f32, tag="cidxf")
                nc.vector.tensor_scalar(cidx_f, inv_f, float(R), consts[:, 26:27],
                                        mybir.AluOpType.mult, ADD)
                nc.vector.tensor_scalar(cidx_f, cidx_f, float(rt * P - R), None, ADD)
                nc.vector.memset(cidx_all[:, rt, :], float(R * H))
                cidxv = p3s.tile([P, E], i16, tag="cidxv")
                nc.vector.tensor_copy(cidxv, cidx_f)
                ind16 = p3s.tile([P, E], i16, tag="ind16")
                nc.vector.tensor_copy(ind16, ind)
                nc.vector.copy_predicated(cidx_all[:, rt, :], ind16, cidxv)

                # spill q_all (two psum halves -> one fp16 tile)
                qall_sb = p3.tile([P, E * HD], fp16, tag="qallsb")
                nc.vector.tensor_copy(qall_sb[:, :E * HD // 2], ps_qh[0])
                nc.vector.tensor_copy(qall_sb[:, E * HD // 2:], ps_qh[1])
                nc.scalar.dma_start(
                    _ap(qall_d, rt * P * E * HD, [[E * HD, P], [1, E * HD]]),
                    qall_sb)

        nc.sync.dma_start(
            qidx_d.ap().rearrange("(rt p) h -> p rt h", p=P), qidx_all)
        nc.scalar.dma_start(
            cidx_d.ap().rearrange("(rt p) e -> p rt e", p=P), cidx_all)

        # wrapped (16-partition interleaved, 8x replicated) index reads for the
        # gathers -- hoisted here so they overlap phases 4-6
        qidx_w = persist.tile([P, R * H // 16], i16)
        for g in range(8):
            nc.sync.dma_start(
                qidx_w[g * 16:(g + 1) * 16, :],
                _ap(qidx_d, 0, [[H, 16], [1, H], [P * H, RT], [16 * H, 8]]))
        cidx_w = persist.tile([P, R * E // 16], i16)
        for g in range(8):
            nc.scalar.dma_start(
                cidx_w[g * 16:(g + 1) * 16, :],
                _ap(cidx_d, 0, [[E, 16], [1, E], [P * E, RT], [16 * E, 8]]))

        ctx_wq.__exit__(None, None, None)
        ctx_xt.__exit__(None, None, None)

        # aux out
        with tc.tile_pool(name="paux", bufs=1) as paux:
            aux_sb = paux.tile([P, 64], f32)
            nc.vector.memset(aux_sb, 0.0)
            nc.vector.tensor_copy(aux_sb[:, 0:E], freq_acc)
            nc.vector.tensor_copy(aux_sb[:, E:2 * E], pacc)
            nc.vector.tensor_copy(aux_sb[:, 2 * E:2 * E + 1], zacc)
            nc.sync.dma_start(aux.ap()[:, :], aux_sb)

        if _STOP_AFTER[0] < 5:
            _stub_outputs(tc, nc, y, aux)
            return
        # ======== phase 5: gather q, build q^T ========
        ctx_qt = tc.tile_pool(name="qt_pool", bufs=1)
        qt_pool = ctx_qt.__enter__()
        qT_sb = qt_pool.tile([P, H, R], fp16)       # gathered q^T per head
        with tc.tile_pool(name="p5", bufs=1) as p5, \
             tc.tile_pool(name="p5ps", bufs=4, space="PSUM") as p5ps:
            q_sel = p5.tile([P, R * H // P, HD], fp16)
            for blk in range(H):
                nc.gpsimd.dma_gather(
                    q_sel[:, blk * (R // P):(blk + 1) * (R // P), :],
                    qall_d.ap(), qidx_w[:, blk * (R // 16):(blk + 1) * (R // 16)],
                    num_idxs=R, num_idxs_reg=R, elem_size=HD)
            with tc.tile_pool(name="p5b", bufs=3) as p5b:
                for h in range(H):
                    for rt in range(RT):
                        q32 = p5b.tile([P, P], f32, tag="q32")
                        nc.vector.tensor_copy(q32, q_sel[:, h * RT + rt, :])
                        ps = p5ps.tile([P, P], f32, tag="qtp")
                        nc.tensor.transpose(ps, q32, ident)
                        nc.vector.tensor_copy(qT_sb[:, h, rt * P:(rt + 1) * P], ps)

        if _STOP_AFTER[0] < 6:
            _stub_outputs(tc, nc, y, aux)
            ctx_qt.__exit__(None, None, None)
            return
        # ======== phase 6: attention ========
        with tc.tile_pool(name="p6e", bufs=2) as p6e, \
             tc.tile_pool(name="p6s", bufs=2) as p6s, \
             tc.tile_pool(name="p6ps", bufs=4, space="PSUM") as p6ps, \
             tc.tile_pool(name="p6po", bufs=2, space="PSUM") as p6po:
            for h in range(H):
                expS = p6e.tile([P, CT, R], fp16, tag="expS")
                for rb in range(2):
                    for ct in range(CT):
                        ps_s = p6ps.tile([P, 512], f32, tag="s")
                        nc.tensor.matmul(
                            ps_s,
                            kvT[:, 0, ct * P:(ct + 1) * P],
                            qT_sb[:, h, rb * 512:(rb + 1) * 512],
                            start=True, stop=True)
                        nc.scalar.activation(
                            expS[:, ct, rb * 512:(rb + 1) * 512], ps_s, Exp,
                            scale=SCALE)
                for rt in range(RT):
                    ps_o = p6po.tile([P, HD + 1], f32, tag="o")
                    for ct in range(CT):
                        nc.tensor.matmul(
                            ps_o, expS[:, ct, rt * P:(rt + 1) * P],
                            v1[:, ct, :],
                            start=(ct == 0), stop=(ct == CT - 1))
                    rd = p6s.tile([P, 1], f32, tag="rd")
                    nc.vector.reciprocal(rd, ps_o[:, HD:HD + 1])
                    g2 = p6s.tile([P, 1], f32, tag="g2")
                    nc.vector.tensor_tensor(g2, gates_sb[:, rt, h:h + 1], rd, MULT)
                    o_sb = p6s.tile([P, HD], fp16, tag="osb")
                    nc.vector.tensor_scalar_mul(o_sb, ps_o[:, 0:HD], g2)
                    nc.sync.dma_start(
                        _ap(o_d, (h * R + rt * P) * HD, [[HD, P], [1, HD]]),
                        o_sb)

        ctx_qt.__exit__(None, None, None)
        if _STOP_AFTER[0] < 7:
            _stub_outputs(tc, nc, y, aux)
            return

        # ======== phase 7+8: gather c^T, combine ========
        with tc.tile_pool(name="p8w", bufs=1) as p8w, \
             tc.tile_pool(name="p8", bufs=2) as p8, \
             tc.tile_pool(name="p8ps", bufs=2, space="PSUM") as p8ps:
            cT = p8w.tile([P, 1, R * E], fp16)
            # column order j = rt*3072 + e*128 + p so combine(rt) only needs
            # its own 4 gather blocks (768 idxs each; <=~1024 descriptors fit
            # the SWDGE carveout)
            GCH = 768
            wo_sb = p8w.tile([P, E, D], fp16)
            nc.scalar.dma_start(wo_sb, wo.ap().rearrange("(e p) n -> p e n", p=P))
            for blk in range(R * E // GCH):
                nc.gpsimd.dma_gather(
                    cT[:, :, blk * GCH:(blk + 1) * GCH],
                    o_d.ap(), cidx_w[:, blk * (GCH // 16):(blk + 1) * (GCH // 16)],
                    num_idxs=GCH, num_idxs_reg=GCH, elem_size=HD, transpose=True)

            # two expert halves: first half's matmuls only need the first 16
            # gather blocks, so they overlap the remaining gathers
            ADD_ = mybir.AluOpType.add
            for rt in range(RT):
                y_sb = p8.tile([P, D], f32, tag="ysb")
                for eh in range(2):
                    ps_y = []
                    for db in range(2):
                        t = p8ps.tile([P, 512], f32, tag=f"y{db}", name=f"psy{eh}{db}")
                        ps_y.append(t)
                    for ei in range(E // 2):
                        e = eh * (E // 2) + ei
                        lhs = cT[:, 0, e * R + rt * P: e * R + (rt + 1) * P]
                        for db in range(2):
                            nc.tensor.matmul(
                                ps_y[db], lhs, wo_sb[:, e, db * 512:(db + 1) * 512],
                                start=(ei == 0), stop=(ei == E // 2 - 1))
                    for db in range(2):
                        if eh == 0:
                            nc.vector.tensor_copy(y_sb[:, db * 512:(db + 1) * 512], ps_y[db])
                        else:
                            nc.vector.tensor_tensor(
                                y_sb[:, db * 512:(db + 1) * 512],
                                y_sb[:, db * 512:(db + 1) * 512], ps_y[db], ADD_)
                nc.scalar.dma_start(y.ap()[rt * P:(rt + 1) * P, :], y_sb)


def _get_nc():
    global _NC_CACHE
    if _NC_CACHE is None:
        _NC_CACHE = build_bass()
    return _NC_CACHE


def _ident_const():
    return np.eye(P, dtype=np.float32)


def _consts_const():
    c = np.zeros((P, 32), dtype=np.float32)
    c[:, 0:E] = np.arange(E, dtype=np.float32)[None, :]
    c[:, 24] = np.arange(P, dtype=np.float32) * E
    c[:, 25] = np.arange(P, dtype=np.float32) * 9
    c[:, 26] = np.arange(P, dtype=np.float32)
    return c


def kernel(x, task_bh, Wg, Wq, Wo, Wkv, bkv):
    from concourse.bass_utils import run_bass_kernel_spmd

    x = np.asarray(x, dtype=np.float32)
    task = np.asarray(task_bh).astype(np.int64)
    Wg = np.asarray(Wg, dtype=np.float32)
    Wq = np.asarray(Wq, dtype=np.float32)
    Wo = np.asarray(Wo, dtype=np.float32)
    Wkv = np.asarray(Wkv, dtype=np.float32)
    bkv = np.asarray(bkv, dtype=np.float32)

    wq_flat = np.ascontiguousarray(Wq.transpose(1, 0, 2).reshape(D, E * HD)).astype(np.float16)
    wo_bf = np.ascontiguousarray(Wo.reshape(E * HD, D)).astype(np.float16)

    in_maps = []
    for core in range(8):
        b, half = core // 2, core % 2
        xb = x[b]
        x_rot = np.ascontiguousarray(
            np.concatenate([xb[half * R:(half + 1) * R],
                            xb[(1 - half) * R:(2 - half) * R]], axis=0))
        in_maps.append({
            "x_ctx": x_rot,
            "wg": np.ascontiguousarray(Wg[task[b]]),
            "wq": wq_flat,
            "wo": wo_bf,
            "wkv": Wkv.astype(np.float16),
            "bkv": bkv,
            "ident": _ident_const(),
            "consts": _consts_const(),
        })

    nc = _get_nc()
    res = run_bass_kernel_spmd(nc, in_maps, core_ids=list(range(8)))
    outs = res.results

    yfull = np.empty((B, N, D), dtype=np.float32)
    freq_cnt = np.zeros(E, dtype=np.float64)
    psum = np.zeros(E, dtype=np.float64)
    zsum = 0.0
    for core in range(8):
        b, half = core // 2, core % 2
        yfull[b, half * R:(half + 1) * R] = outs[core]["y"]
        a = outs[core]["aux"]
        freq_cnt += a[:, 0:E].sum(axis=0)
        psum += a[:, E:2 * E].sum(axis=0)
        zsum += a[:, 2 * E].sum()
    denom = B * N
    freq = freq_cnt / denom
    pmean = psum / denom
    switch = SWITCHLOSS * E * float(np.dot(freq, pmean))
    zl = ZLOSS * zsum / denom
    aux_loss = np.float32(switch + zl)
    return yfull, aux_loss
